# revision 2
# baseline (speedup 1.0000x reference)
"""AUCM loss (pairwise squared-hinge-free AUC surrogate) Trainium2 kernel.

Reference computes, for logits/targets [B=1024, C=128]:
    probs = sigmoid(logits)
    num[c]  = sum_{i,j} softplus(p_j - p_i) * pos[i,c] * neg[j,c]
    loss    = masked mean over classes of num[c] / (n_pos[c]*n_neg[c])

Direct evaluation is O(B^2 C) = 134M softplus terms.  Since probs in (0,1),
the pairwise argument t = p_j - p_i lies in (-1,1) where softplus is analytic
(nearest complex singularity at +-i*pi), so a degree-8 Chebyshev fit of
softplus on [-1,1] is accurate to 6.4e-9.  Expanding (b-a)^k binomially turns
the pairwise sum into per-class weighted power sums ("moments"):

    num[c] = sum_{m+n<=8} Bm[m,n] * Sn[m,c] * Sp[n,c]
    Sp[n,c] = sum_i pos[i,c] * a_i^n,   Sn[m,c] = sum_j neg[j,c] * a_j^m

with a_i = tanh(logits_i/2) = 2*(probs_i - 0.5) computed in ONE activation op
(coefficients pre-scaled by 2^-k on the host).  This is O(B C D) work.

Sharding: data-parallel over the class axis C (16 classes/core, batch
replicated per the pairwise structure).  Each core returns its partial
(sum of per-class means, count of valid classes); the host unshard step sums
the 8 partial pairs and forms the final scalar.

Layout per core: SBUF tile [128p, 128f] where partition p holds batch rows
8p..8p+7 (f = ib*16 + c) -- a fully contiguous DMA of the host-sliced
[1024,16] class shard.  Moment power chain runs on DVE; the batch reduction
runs on the tensor engine as accumulating matmuls with one-hot stationary
matrices E_k (row k of PSUM[9, 32] collects moment k for both masks), so the
DVE only does the 8 chain multiplies.  The bilinear combination is one more
small matmul with the coefficient matrix, then a handful of [1,16] vector ops.
"""

import os
import sys
from math import comb

import numpy as np

for _p in ("/opt/trn_rl_repo", "/root/.axon_site/_ro/trn_rl_repo"):
    if os.path.isdir(_p) and _p not in sys.path:
        sys.path.append(_p)

import concourse.bacc as bacc
import concourse.bass as bass
import concourse.mybir as mybir
import concourse.tile as tile
from concourse import bass_utils

B_FULL, C_FULL = 1024, 128
N_CORES = 8
C_SHARD = C_FULL // N_CORES          # 16 classes per core
P = 128                              # partitions
IB = B_FULL // P                     # 8 batch rows folded per partition
DEG = 8
NMOM = DEG + 1                       # 9 moments (k = 0..8)

# Degree-8 Chebyshev fit of softplus on [-1, 1] (max err 6.4e-9), monomial basis.
A_COEF = np.array(
    [0.6931471805599451, 0.5, 0.12499993751130437, 0.0,
     -0.005207494902105385, 0.0, 0.00034415233029677747, 0.0,
     -2.209438759965319e-05],
    dtype=np.float64,
)


def _host_consts():
    # moments are of a' = tanh(x/2) = 2*(p - 0.5); rescale poly coeffs by 2^-k
    alpha = A_COEF / (2.0 ** np.arange(NMOM))
    bm = np.zeros((NMOM, NMOM))
    for m in range(NMOM):
        for n in range(NMOM - m):
            bm[m, n] = alpha[m + n] * comb(m + n, m) * ((-1.0) ** n)
    # eb: one-hot stationary blocks, flattened [k*NMOM + m] = (m == k)
    eb = np.eye(NMOM, dtype=np.float32).reshape(1, NMOM * NMOM)
    # bt[n, m] = bm[m, n]  (lhsT layout for H = Bm @ Sp)
    bt = np.ascontiguousarray(bm.T.astype(np.float32))
    return eb, bt


def build_bass():
    """Build (and cache) the compiled SPMD Bass program."""
    f32 = mybir.dt.float32
    nc = bacc.Bacc("TRN2", target_bir_lowering=False, debug=False)

    lg = nc.dram_tensor("logits", [B_FULL, C_SHARD], f32, kind="ExternalInput")
    tg = nc.dram_tensor("targets", [B_FULL, C_SHARD], f32, kind="ExternalInput")
    eb = nc.dram_tensor("eb", [1, NMOM * NMOM], f32, kind="ExternalInput")
    bt = nc.dram_tensor("bt", [NMOM, NMOM], f32, kind="ExternalInput")
    out_d = nc.dram_tensor("out", [1, 2], f32, kind="ExternalOutput")

    mult = mybir.AluOpType.mult
    add = mybir.AluOpType.add
    is_gt = mybir.AluOpType.is_gt

    with tile.TileContext(nc) as tc:
        with (
            tc.tile_pool(name="sb", bufs=1) as pool,
            tc.tile_pool(name="ps", bufs=1, space="PSUM") as pps,
        ):
            # ---- inputs -> SBUF (contiguous loads) --------------------------
            X = pool.tile([P, IB * C_SHARD], f32, tag="X")
            TGs = pool.tile([P, IB * C_SHARD], f32, tag="TGs")
            nc.sync.dma_start(
                out=X[:, :], in_=lg.ap().rearrange("(p q) c -> p (q c)", p=P)
            )
            nc.sync.dma_start(
                out=TGs[:, :], in_=tg.ap().rearrange("(p q) c -> p (q c)", p=P)
            )
            E = pool.tile([P, NMOM * NMOM], f32, tag="E")
            nc.gpsimd.dma_start(out=E[:, :], in_=eb.ap().to_broadcast([P, NMOM * NMOM]))
            BT = pool.tile([NMOM, NMOM], f32, tag="BT")
            nc.sync.dma_start(out=BT[:, :], in_=bt.ap())

            # ---- a' = tanh(x/2) --------------------------------------------
            A = pool.tile([P, IB * C_SHARD], f32, tag="A")
            nc.scalar.activation(
                A[:, :], X[:, :], mybir.ActivationFunctionType.Tanh, scale=0.5
            )

            # ---- masked power chain W[k][p, s, f] = mask_s * a'^k ----------
            W = [
                pool.tile([P, 2, IB * C_SHARD], f32, tag=f"W{k}", name=f"W{k}")
                for k in range(NMOM)
            ]
            nc.vector.tensor_copy(W[0][:, 0, :], TGs[:, :])                 # pos
            nc.vector.tensor_scalar(W[0][:, 1, :], TGs[:, :], -1.0, 1.0,
                                    op0=mult, op1=add)                      # neg = 1-t
            nc.vector.tensor_mul(W[1][:, 0, :], W[0][:, 0, :], A[:, :])
            nc.vector.tensor_sub(W[1][:, 1, :], A[:, :], W[1][:, 0, :])     # (1-t)*a = a - t*a
            for k in range(2, NMOM):
                h, o = k // 2, k - k // 2
                nc.vector.tensor_mul(W[k][:, :, :], W[h][:, :, :], W[o][:, :, :])

            # ---- batch reduction on PE: PS[k, s*16+c] = sum_b W[k][b,s,c] --
            PS = pps.tile([NMOM, 2 * C_SHARD], f32, tag="PS")
            n_mm = NMOM * IB
            i_mm = 0
            for k in range(NMOM):
                lhsT = E[:, k * NMOM:(k + 1) * NMOM]        # one-hot col k
                for ib in range(IB):
                    rhs = W[k][:, :, ib * C_SHARD:(ib + 1) * C_SHARD]
                    nc.tensor.matmul(
                        PS[:, :], lhsT, rhs,
                        start=(i_mm == 0), stop=(i_mm == n_mm - 1),
                    )
                    i_mm += 1

            S = pool.tile([NMOM, 2 * C_SHARD], f32, tag="S")
            nc.vector.tensor_copy(S[:, :], PS[:, :])
            Sp = S[:, 0:C_SHARD]
            Sn = S[:, C_SHARD:2 * C_SHARD]

            # ---- bilinear combination num[c] = sum_m Sn[m,c] * (Bm@Sp)[m,c]
            H = pps.tile([NMOM, C_SHARD], f32, tag="H")
            nc.tensor.matmul(H[:, :], BT[:, :], Sp, start=True, stop=True)
            G = pool.tile([NMOM, C_SHARD], f32, tag="G")
            nc.vector.tensor_mul(G[:, :], Sn, H[:, :])
            NUM = pps.tile([1, C_SHARD], f32, tag="NUM")
            nc.tensor.matmul(NUM[:, :], E[0:NMOM, 0:1], G[:, :], start=True, stop=True)

            # ---- per-class mean + validity ---------------------------------
            cnt = pool.tile([1, C_SHARD], f32, tag="cnt")
            nc.vector.tensor_mul(cnt[:, :], S[0:1, 0:C_SHARD], S[0:1, C_SHARD:2 * C_SHARD])
            valid = pool.tile([1, C_SHARD], f32, tag="valid")
            nc.vector.tensor_scalar(valid[:, :], cnt[:, :], 0.5, None, op0=is_gt)
            safe = pool.tile([1, C_SHARD], f32, tag="safe")
            nc.vector.tensor_scalar_max(safe[:, :], cnt[:, :], 1.0)
            rec = pool.tile([1, C_SHARD], f32, tag="rec")
            nc.vector.reciprocal(rec[:, :], safe[:, :])
            meanv = pool.tile([1, C_SHARD], f32, tag="meanv")
            nc.vector.tensor_mul(meanv[:, :], NUM[:, :], rec[:, :])
            meanm = pool.tile([1, C_SHARD], f32, tag="meanm")
            nc.vector.tensor_mul(meanm[:, :], meanv[:, :], valid[:, :])

            OUT = pool.tile([1, 2], f32, tag="OUT")
            nc.vector.reduce_sum(OUT[:, 0:1], meanm[:, :], axis=mybir.AxisListType.X)
            nc.vector.reduce_sum(OUT[:, 1:2], valid[:, :], axis=mybir.AxisListType.X)
            nc.sync.dma_start(out=out_d.ap(), in_=OUT[:, :])

    nc.compile()
    return nc


_CACHE = {}


def _compiled():
    if "nc" not in _CACHE:
        _CACHE["nc"] = build_bass()
    return _CACHE["nc"]


def make_in_maps(logits, targets):
    eb, bt = _host_consts()
    logits = np.ascontiguousarray(logits, dtype=np.float32)
    targets = np.ascontiguousarray(targets, dtype=np.float32)
    in_maps = []
    for k in range(N_CORES):
        sl = slice(k * C_SHARD, (k + 1) * C_SHARD)
        in_maps.append({
            "logits": np.ascontiguousarray(logits[:, sl]),
            "targets": np.ascontiguousarray(targets[:, sl]),
            "eb": eb,
            "bt": bt,
        })
    return in_maps


def combine_outputs(core_outs):
    """core_outs: list of [1,2] arrays -> scalar loss (matches reference)."""
    f32 = np.float32
    parts = np.stack([np.asarray(o, f32).reshape(2) for o in core_outs])
    sums = parts[:, 0].sum(dtype=f32)
    vc = parts[:, 1].sum(dtype=f32)
    if vc > 0:
        loss = f32(sums / max(vc, f32(1.0)))
    else:
        loss = f32(0.0)
    return np.asarray(loss, dtype=np.float32)


def kernel(logits, targets):
    nc = _compiled()
    in_maps = make_in_maps(logits, targets)
    res = bass_utils.run_bass_kernel_spmd(nc, in_maps, core_ids=list(range(N_CORES)))
    return combine_outputs([r["out"] for r in res.results])


# revision 6
# speedup vs baseline: 1.1805x; 1.1805x over previous
"""AUCM loss (pairwise softplus AUC surrogate) Trainium2 kernel.

Reference, for logits/targets [B=1024, C=128]:
    probs = sigmoid(logits)
    num[c] = sum_{i,j} softplus(p_j - p_i) * pos[i,c] * neg[j,c]
    loss   = masked mean over classes of num[c] / (n_pos[c]*n_neg[c])

Direct evaluation is O(B^2 C) = 134M softplus terms.  Since probs in (0,1),
the pairwise argument lies in (-1,1) where softplus is analytic (nearest
complex singularity at +-i*pi), so a degree-6 Chebyshev fit of softplus on
[-1,1] (max err 3.3e-7) turns the pairwise sum into per-class weighted power
sums ("moments") via the binomial expansion:

    num[c] = sum_{m+n<=6} Bm[m,n] * Sn[m,c] * Sp[n,c]
    Sp[n,c] = sum_i pos[i,c] a_i^n,  Sn[m,c] = sum_j neg[j,c] a_j^m

with a_i = tanh(logits_i/2) = 2*(probs_i - 0.5) computed in ONE activation op
(coefficients pre-scaled by 2^-k on the host).  O(B C D) work.

Sharding: data-parallel over the class axis (16 classes/core, batch
replicated, per the pairwise structure).  Each core returns its partial
(sum of per-class means, count of valid classes); the host unshard step sums
the 8 partial pairs and forms the final scalar exactly as the reference does.

Per-core dataflow ([128p, 128f] tile, partition p holds batch rows 8p..8p+7):
  - DVE builds the masked power tiles W_k[p, s, ibc] = mask_s * a^k via 5
    tensor_tensor multiplies (square/product chain).
  - PE does the batch reduction AND the coefficient combination in one
    accumulating matmul group: stationary for moment k is [128, 14] with
    columns j<7 = Bm[j,k] (accumulates H = Bm @ Sp directly) and columns
    j>=7 = one-hot k (collects the raw moments); PSUM [14, 256] accumulates
    over k.
  - One DVE segmented reduce folds the 8-way batch-fold axis: SB [14, 2*16].
  - Tail: G = Sn (.) H, num = ones @ G, per-class mean + validity masking,
    and a [1,2] result (sum of means, valid count) DMA'd out.
"""

import os
import sys
from math import comb

import numpy as np

for _p in ("/opt/trn_rl_repo", "/root/.axon_site/_ro/trn_rl_repo"):
    if os.path.isdir(_p) and _p not in sys.path:
        sys.path.append(_p)

import concourse.bacc as bacc
import concourse.bass as bass
import concourse.mybir as mybir
import concourse.tile as tile
from concourse import bass_utils

B_FULL, C_FULL = 1024, 128
N_CORES = 8
C_SHARD = C_FULL // N_CORES          # 16 classes per core
P = 128                              # partitions
IB = B_FULL // P                     # 8 batch rows folded per partition
DEG = 6
NMOM = DEG + 1                       # 7 moments (k = 0..6)
NST = 2 * NMOM                       # stationary columns (H part + raw part)
ONES_COL = NMOM * NST                # all-ones column (final sum lhsT)
SEL_COL = ONES_COL + 1               # 7-wide row-selection block (rows 7..13)
CN_COLS = SEL_COL + NMOM + 1         # + pad

# Degree-6 Chebyshev fit of softplus on [-1, 1] (max err 3.3e-7), monomial.
A_COEF = np.array(
    [0.6931471805599451, 0.5, 0.12499748720039783, 0.0,
     -0.005188028447445448, 0.0, 0.0003053804886608954],
    dtype=np.float64,
)


def _host_consts():
    # moments are of a = tanh(x/2) = 2*(p - 0.5); rescale poly coeffs by 2^-k
    alpha = A_COEF / (2.0 ** np.arange(NMOM))
    bm = np.zeros((NMOM, NMOM))
    for m in range(NMOM):
        for n in range(NMOM - m):
            bm[m, n] = alpha[m + n] * comb(m + n, m) * ((-1.0) ** n)
    row = np.zeros(CN_COLS, np.float32)
    for k in range(NMOM):
        row[k * NST:k * NST + NMOM] = bm[:, k]       # H-part: col j = Bm[j, k]
        row[k * NST + NMOM + k] = 1.0                # raw part: one-hot k
    row[ONES_COL] = 1.0                              # ones column (final sum)
    cn = np.ascontiguousarray(np.broadcast_to(row, (P, CN_COLS)), np.float32)
    # row-selection block: lhsT [14, 7] picking rows 7..13 down to 0..6
    for m in range(NMOM):
        cn[NMOM + m, SEL_COL + m] = 1.0
    return cn


def build_bass():
    f32 = mybir.dt.float32
    nc = bacc.Bacc("TRN2", target_bir_lowering=False, debug=False)

    lg = nc.dram_tensor("logits", [B_FULL, C_SHARD], f32, kind="ExternalInput")
    tg = nc.dram_tensor("targets", [B_FULL, C_SHARD], f32, kind="ExternalInput")
    cn = nc.dram_tensor("cn", [P, CN_COLS], f32, kind="ExternalInput")
    out_d = nc.dram_tensor("out", [1, 2], f32, kind="ExternalOutput")

    mult = mybir.AluOpType.mult
    add = mybir.AluOpType.add
    is_gt = mybir.AluOpType.is_gt

    with tile.TileContext(nc) as tc:
        with (
            tc.tile_pool(name="sb", bufs=1) as pool,
            tc.tile_pool(name="ps", bufs=1, space="PSUM") as pps,
        ):
            # ---- inputs -> SBUF (contiguous loads, two HWDGE rings) --------
            X = pool.tile([P, IB * C_SHARD], f32, tag="X")
            TGs = pool.tile([P, IB * C_SHARD], f32, tag="TGs")
            CN = pool.tile([P, CN_COLS], f32, tag="CN")
            nc.sync.dma_start(
                out=X[:, :], in_=lg.ap().rearrange("(p q) c -> p (q c)", p=P)
            )
            nc.scalar.dma_start(
                out=TGs[:, :], in_=tg.ap().rearrange("(p q) c -> p (q c)", p=P)
            )
            nc.sync.dma_start(out=CN[:, :], in_=cn.ap())

            # ---- a = tanh(x/2) ---------------------------------------------
            A = pool.tile([P, IB * C_SHARD], f32, tag="A")
            nc.scalar.activation(
                A[:, :], X[:, :], mybir.ActivationFunctionType.Tanh, scale=0.5
            )

            # ---- masked power tiles W_k[p, s, ibc] = mask_s * a^k ----------
            W = [
                pool.tile([P, 2, IB * C_SHARD], f32, tag=f"W{k}", name=f"W{k}")
                for k in range(NMOM)
            ]
            nc.vector.tensor_copy(W[0][:, 0, :], TGs[:, :])             # pos
            nc.vector.tensor_scalar(W[0][:, 1, :], TGs[:, :], -1.0, 1.0,
                                    op0=mult, op1=add)                  # 1 - t
            nc.vector.tensor_mul(W[1][:, 0, :], W[0][:, 0, :], A[:, :])
            nc.vector.tensor_sub(W[1][:, 1, :], A[:, :], W[1][:, 0, :])
            nc.vector.tensor_mul(W[2][:, :, :], W[1][:, :, :], W[1][:, :, :])
            nc.vector.tensor_mul(W[3][:, :, :], W[1][:, :, :], W[2][:, :, :])
            nc.vector.tensor_mul(W[4][:, :, :], W[2][:, :, :], W[2][:, :, :])
            nc.vector.tensor_mul(W[5][:, :, :], W[2][:, :, :], W[3][:, :, :])
            nc.vector.tensor_mul(W[6][:, :, :], W[3][:, :, :], W[3][:, :, :])

            # ---- PE: batch-sum + coefficient combination, one MM group ----
            # PSA[j, (s ib c)] = sum_k ST[k][j] * colsum_p(W_k)
            PSA = pps.tile([NST, 2 * IB * C_SHARD], f32, tag="PSA")
            for k in range(NMOM):
                nc.tensor.matmul(
                    PSA[:, :],
                    CN[:, k * NST:(k + 1) * NST],
                    W[k][:, :, :],
                    start=(k == 0),
                    stop=(k == NMOM - 1),
                )

            # ---- fold the ib axis: SB[j, s*16+c] ---------------------------
            SB = pool.tile([NST, 2 * C_SHARD], f32, tag="SB")
            nc.vector.reduce_sum(
                SB[:, :].rearrange("p (s c) -> p s c", s=2),
                PSA[:, :].rearrange("p (s ib c) -> p s c ib", s=2, ib=IB),
                axis=mybir.AxisListType.X,
            )
            # rows 0..6 cols 0:16   = H[m,c] = sum_n Bm[m,n] Sp[n,c]
            # rows 7..13            = [Sp[k] | Sn[k]]

            # ---- relocate raw rows 7..13 to partitions 0..6 (matmul moves
            # partitions; DVE slices must start at partition 0/32/64/96) ----
            RAW = pps.tile([NMOM, 2 * C_SHARD], f32, tag="RAW")
            nc.tensor.matmul(
                RAW[:, :], CN[0:NST, SEL_COL:SEL_COL + NMOM], SB[:, :],
                start=True, stop=True,
            )
            RAWS = pool.tile([NMOM, 2 * C_SHARD], f32, tag="RAWS")
            nc.vector.tensor_copy(RAWS[:, :], RAW[:, :])

            # ---- num[c] = sum_m Sn[m,c] * H[m,c] ---------------------------
            G = pool.tile([NMOM, C_SHARD], f32, tag="G")
            nc.vector.tensor_mul(
                G[:, :], RAWS[:, C_SHARD:2 * C_SHARD], SB[0:NMOM, 0:C_SHARD]
            )
            NUM = pps.tile([1, C_SHARD], f32, tag="NUM")
            nc.tensor.matmul(
                NUM[:, :], CN[0:NMOM, ONES_COL:ONES_COL + 1], G[:, :],
                start=True, stop=True,
            )

            # ---- per-class mean + validity ---------------------------------
            RES = pool.tile([1, 2, C_SHARD], f32, tag="RES")
            cnt = pool.tile([1, C_SHARD], f32, tag="cnt")
            nc.vector.tensor_mul(
                cnt[:, :], RAWS[0:1, 0:C_SHARD], RAWS[0:1, C_SHARD:2 * C_SHARD]
            )
            nc.vector.tensor_scalar(RES[:, 1, :], cnt[:, :], 0.5, None, op0=is_gt)
            safe = pool.tile([1, C_SHARD], f32, tag="safe")
            nc.vector.tensor_scalar_max(safe[:, :], cnt[:, :], 1.0)
            rec = pool.tile([1, C_SHARD], f32, tag="rec")
            nc.vector.reciprocal(rec[:, :], safe[:, :])
            mask2 = pool.tile([1, C_SHARD], f32, tag="mask2")
            nc.vector.tensor_mul(mask2[:, :], rec[:, :], RES[:, 1, :])
            nc.vector.tensor_mul(RES[:, 0, :], NUM[:, :], mask2[:, :])

            OUT = pool.tile([1, 2], f32, tag="OUT")
            nc.vector.reduce_sum(OUT[:, :], RES[:, :, :], axis=mybir.AxisListType.X)
            nc.sync.dma_start(out=out_d.ap(), in_=OUT[:, :])

    nc.compile()
    return nc


_CACHE = {}


def _compiled():
    if "nc" not in _CACHE:
        _CACHE["nc"] = build_bass()
    return _CACHE["nc"]


def make_in_maps(logits, targets):
    cn = _host_consts()
    logits = np.ascontiguousarray(logits, dtype=np.float32)
    targets = np.ascontiguousarray(targets, dtype=np.float32)
    in_maps = []
    for k in range(N_CORES):
        sl = slice(k * C_SHARD, (k + 1) * C_SHARD)
        in_maps.append({
            "logits": np.ascontiguousarray(logits[:, sl]),
            "targets": np.ascontiguousarray(targets[:, sl]),
            "cn": cn,
        })
    return in_maps


def combine_outputs(core_outs):
    """core_outs: list of [1,2] arrays -> scalar loss (matches reference)."""
    f32 = np.float32
    parts = np.stack([np.asarray(o, f32).reshape(2) for o in core_outs])
    sums = parts[:, 0].sum(dtype=f32)
    vc = parts[:, 1].sum(dtype=f32)
    if vc > 0:
        loss = f32(sums / max(vc, f32(1.0)))
    else:
        loss = f32(0.0)
    return np.asarray(loss, dtype=np.float32)


def kernel(logits, targets):
    nc = _compiled()
    in_maps = make_in_maps(logits, targets)
    res = bass_utils.run_bass_kernel_spmd(nc, in_maps, core_ids=list(range(N_CORES)))
    return combine_outputs([r["out"] for r in res.results])


# revision 9
# speedup vs baseline: 1.3132x; 1.1123x over previous
"""AUCM loss (pairwise softplus AUC surrogate) Trainium2 kernel.

Reference, for logits/targets [B=1024, C=128]:
    probs = sigmoid(logits)
    num[c] = sum_{i,j} softplus(p_j - p_i) * pos[i,c] * neg[j,c]
    loss   = masked mean over classes of num[c] / (n_pos[c]*n_neg[c])

Direct evaluation is O(B^2 C) = 134M softplus terms.  Since probs in (0,1),
the pairwise argument lies in (-1,1) where softplus is analytic (nearest
complex singularity at +-i*pi), so a degree-6 Chebyshev fit of softplus on
[-1,1] (max err 3.3e-7) turns the pairwise sum into per-class weighted power
sums ("moments") via the binomial expansion:

    num[c] = sum_{m+n<=6} Bm[m,n] * Sn[m,c] * Sp[n,c]
    Sp[n,c] = sum_i pos[i,c] a_i^n,  Sn[m,c] = sum_j neg[j,c] a_j^m

with a_i = tanh(logits_i/2) = 2*(probs_i - 0.5) computed in ONE activation op
(coefficients pre-scaled by 2^-k on the host).  O(B C D) work.

Sharding: data-parallel over the class axis (16 classes/core, batch
replicated, per the pairwise structure).  Each core returns its partial
(sum of per-class means, count of valid classes); the host unshard step sums
the 8 partial pairs and forms the final scalar exactly as the reference does.

Per-core dataflow ([128p, 128f] tile, partition p holds batch rows 8p..8p+7):
  - DVE builds the masked power tiles W_k[p, s, ibc] = mask_s * a^k via 5
    tensor_tensor multiplies (square/product chain).
  - PE does the batch reduction AND the coefficient combination in one
    accumulating matmul group: stationary for moment k is [128, 14] with
    columns j<7 = Bm[j,k] (accumulates H = Bm @ Sp directly) and columns
    j>=7 = one-hot k (collects the raw moments); PSUM [14, 256] accumulates
    over k.
  - One DVE segmented reduce folds the 8-way batch-fold axis: SB [14, 2*16].
  - Tail: G = Sn (.) H, num = ones @ G, per-class mean + validity masking,
    and a [1,2] result (sum of means, valid count) DMA'd out.
"""

import os
import sys
from math import comb

import numpy as np

for _p in ("/opt/trn_rl_repo", "/root/.axon_site/_ro/trn_rl_repo"):
    if os.path.isdir(_p) and _p not in sys.path:
        sys.path.append(_p)

import concourse.bacc as bacc
import concourse.bass as bass
import concourse.mybir as mybir
import concourse.tile as tile
from concourse import bass_utils

B_FULL, C_FULL = 1024, 128
N_CORES = 8
C_SHARD = C_FULL // N_CORES          # 16 classes per core
P = 128                              # partitions
IB = B_FULL // P                     # 8 batch rows folded per partition
DEG = 6
NMOM = DEG + 1                       # 7 moments (k = 0..6)
NST = 2 * NMOM                       # stationary columns (H part + raw part)
ONES_COL = NMOM * NST                # all-ones column (final sum lhsT)
SEL_COL = ONES_COL + 1               # 7-wide row-selection block (rows 7..13)
CN_COLS = SEL_COL + NMOM + 1         # + pad

# Degree-6 Chebyshev fit of softplus on [-1, 1] (max err 3.3e-7), monomial.
A_COEF = np.array(
    [0.6931471805599451, 0.5, 0.12499748720039783, 0.0,
     -0.005188028447445448, 0.0, 0.0003053804886608954],
    dtype=np.float64,
)


def _host_consts():
    # moments are of a = tanh(x/2) = 2*(p - 0.5); rescale poly coeffs by 2^-k
    alpha = A_COEF / (2.0 ** np.arange(NMOM))
    bm = np.zeros((NMOM, NMOM))
    for m in range(NMOM):
        for n in range(NMOM - m):
            bm[m, n] = alpha[m + n] * comb(m + n, m) * ((-1.0) ** n)
    row = np.zeros(CN_COLS, np.float32)
    for k in range(NMOM):
        row[k * NST:k * NST + NMOM] = bm[:, k]       # H-part: col j = Bm[j, k]
        row[k * NST + NMOM + k] = 1.0                # raw part: one-hot k
    row[ONES_COL] = 1.0                              # ones column (final sum)
    cn = np.ascontiguousarray(np.broadcast_to(row, (P, CN_COLS)), np.float32)
    # row-selection block: lhsT [14, 7] picking rows 7..13 down to 0..6
    for m in range(NMOM):
        cn[NMOM + m, SEL_COL + m] = 1.0
    return cn


def build_bass():
    f32 = mybir.dt.float32
    nc = bacc.Bacc("TRN2", target_bir_lowering=False, debug=False)

    lg = nc.dram_tensor("logits", [B_FULL, C_SHARD], f32, kind="ExternalInput")
    tg = nc.dram_tensor("targets", [B_FULL, C_SHARD], f32, kind="ExternalInput")
    cn = nc.dram_tensor("cn", [P, CN_COLS], f32, kind="ExternalInput")
    out_d = nc.dram_tensor("out", [1, 2], f32, kind="ExternalOutput")

    mult = mybir.AluOpType.mult
    add = mybir.AluOpType.add
    is_gt = mybir.AluOpType.is_gt

    with tile.TileContext(nc) as tc:
        with (
            tc.tile_pool(name="sb", bufs=1) as pool,
            tc.tile_pool(name="ps", bufs=1, space="PSUM") as pps,
        ):
            # ---- PE warmup: dummy matmuls during the input-DMA window so
            # the HAM clock gate is at 2.4 GHz when the real matmuls arrive --
            WU = pool.tile([P, 256], mybir.dt.bfloat16, tag="WU")
            nc.vector.memset(WU[:, :], 0.0)
            DPS = pps.tile([1, 256], f32, tag="DPS")
            for _ in range(20):
                nc.tensor.matmul(DPS[:, :], WU[:, 0:1], WU[:, :],
                                 start=True, stop=True)

            # ---- inputs -> SBUF (contiguous loads, two HWDGE rings) --------
            X = pool.tile([P, IB * C_SHARD], f32, tag="X")
            TGs = pool.tile([P, IB * C_SHARD], f32, tag="TGs")
            CN = pool.tile([P, CN_COLS], f32, tag="CN")
            nc.sync.dma_start(
                out=X[:, :], in_=lg.ap().rearrange("(p q) c -> p (q c)", p=P)
            )
            nc.scalar.dma_start(
                out=TGs[:, :], in_=tg.ap().rearrange("(p q) c -> p (q c)", p=P)
            )
            nc.sync.dma_start(out=CN[:, :], in_=cn.ap())

            # ---- a = tanh(x/2) ---------------------------------------------
            A = pool.tile([P, IB * C_SHARD], f32, tag="A")
            nc.scalar.activation(
                A[:, :], X[:, :], mybir.ActivationFunctionType.Tanh, scale=0.5
            )

            # ---- masked power tiles W_k[p, s, ibc] = mask_s * a^k ----------
            W = [
                pool.tile([P, 2, IB * C_SHARD], f32, tag=f"W{k}", name=f"W{k}")
                for k in range(NMOM)
            ]
            nc.vector.tensor_copy(W[0][:, 0, :], TGs[:, :])             # pos
            nc.vector.tensor_scalar(W[0][:, 1, :], TGs[:, :], -1.0, 1.0,
                                    op0=mult, op1=add)                  # 1 - t
            nc.vector.tensor_mul(W[1][:, 0, :], W[0][:, 0, :], A[:, :])
            nc.vector.tensor_sub(W[1][:, 1, :], A[:, :], W[1][:, 0, :])
            nc.vector.tensor_mul(W[2][:, :, :], W[1][:, :, :], W[1][:, :, :])
            nc.vector.tensor_mul(W[3][:, :, :], W[1][:, :, :], W[2][:, :, :])
            nc.vector.tensor_mul(W[4][:, :, :], W[2][:, :, :], W[2][:, :, :])
            nc.vector.tensor_mul(W[5][:, :, :], W[2][:, :, :], W[3][:, :, :])
            nc.vector.tensor_mul(W[6][:, :, :], W[3][:, :, :], W[3][:, :, :])

            # ---- PE: batch-sum + coefficient combination, one MM group ----
            # PSA[j, (s ib c)] = sum_k ST[k][j] * colsum_p(W_k)
            PSA = pps.tile([NST, 2 * IB * C_SHARD], f32, tag="PSA")
            for k in range(NMOM):
                nc.tensor.matmul(
                    PSA[:, :],
                    CN[:, k * NST:(k + 1) * NST],
                    W[k][:, :, :],
                    start=(k == 0),
                    stop=(k == NMOM - 1),
                )

            # ---- fold the ib axis: SB[j, s*16+c] ---------------------------
            SB = pool.tile([NST, 2 * C_SHARD], f32, tag="SB")
            nc.vector.reduce_sum(
                SB[:, :].rearrange("p (s c) -> p s c", s=2),
                PSA[:, :].rearrange("p (s ib c) -> p s c ib", s=2, ib=IB),
                axis=mybir.AxisListType.X,
            )
            # rows 0..6 cols 0:16   = H[m,c] = sum_n Bm[m,n] Sp[n,c]
            # rows 7..13            = [Sp[k] | Sn[k]]

            # ---- relocate raw rows 7..13 to partitions 0..6 (matmul moves
            # partitions; DVE slices must start at partition 0/32/64/96) ----
            RAW = pps.tile([NMOM, 2 * C_SHARD], f32, tag="RAW")
            nc.tensor.matmul(
                RAW[:, :], CN[0:NST, SEL_COL:SEL_COL + NMOM], SB[:, :],
                start=True, stop=True,
            )

            # ---- num[c] = sum_m Sn[m,c] * H[m,c] ---------------------------
            G = pool.tile([NMOM, C_SHARD], f32, tag="G")
            nc.vector.tensor_mul(
                G[:, :], RAW[:, C_SHARD:2 * C_SHARD], SB[0:NMOM, 0:C_SHARD]
            )
            NUM = pps.tile([1, C_SHARD], f32, tag="NUM")
            nc.tensor.matmul(
                NUM[:, :], CN[0:NMOM, ONES_COL:ONES_COL + 1], G[:, :],
                start=True, stop=True,
            )

            # ---- per-class mean + validity ---------------------------------
            # n_neg = B - n_pos exactly, so cnt = Sp0 * (B - Sp0) needs only
            # one PSUM read per op (TT can't take two PSUM operands).
            RES = pool.tile([1, 2, C_SHARD], f32, tag="RES")
            nneg = pool.tile([1, C_SHARD], f32, tag="nneg")
            nc.vector.tensor_scalar(nneg[:, :], RAW[0:1, 0:C_SHARD],
                                    -1.0, float(B_FULL), op0=mult, op1=add)
            cnt = pool.tile([1, C_SHARD], f32, tag="cnt")
            nc.vector.tensor_mul(cnt[:, :], nneg[:, :], RAW[0:1, 0:C_SHARD])
            nc.vector.tensor_scalar(RES[:, 1, :], cnt[:, :], 0.5, None, op0=is_gt)
            safe = pool.tile([1, C_SHARD], f32, tag="safe")
            nc.vector.tensor_scalar_max(safe[:, :], cnt[:, :], 1.0)
            rec = pool.tile([1, C_SHARD], f32, tag="rec")
            nc.vector.reciprocal(rec[:, :], safe[:, :])
            mask2 = pool.tile([1, C_SHARD], f32, tag="mask2")
            nc.vector.tensor_mul(mask2[:, :], rec[:, :], RES[:, 1, :])
            nc.vector.tensor_mul(RES[:, 0, :], NUM[:, :], mask2[:, :])

            OUT = pool.tile([1, 2], f32, tag="OUT")
            nc.vector.reduce_sum(OUT[:, :], RES[:, :, :], axis=mybir.AxisListType.X)
            nc.sync.dma_start(out=out_d.ap(), in_=OUT[:, :])

    nc.compile()
    return nc


_CACHE = {}


def _compiled():
    if "nc" not in _CACHE:
        _CACHE["nc"] = build_bass()
    return _CACHE["nc"]


def make_in_maps(logits, targets):
    cn = _host_consts()
    logits = np.ascontiguousarray(logits, dtype=np.float32)
    targets = np.ascontiguousarray(targets, dtype=np.float32)
    in_maps = []
    for k in range(N_CORES):
        sl = slice(k * C_SHARD, (k + 1) * C_SHARD)
        in_maps.append({
            "logits": np.ascontiguousarray(logits[:, sl]),
            "targets": np.ascontiguousarray(targets[:, sl]),
            "cn": cn,
        })
    return in_maps


def combine_outputs(core_outs):
    """core_outs: list of [1,2] arrays -> scalar loss (matches reference)."""
    f32 = np.float32
    parts = np.stack([np.asarray(o, f32).reshape(2) for o in core_outs])
    sums = parts[:, 0].sum(dtype=f32)
    vc = parts[:, 1].sum(dtype=f32)
    if vc > 0:
        loss = f32(sums / max(vc, f32(1.0)))
    else:
        loss = f32(0.0)
    return np.asarray(loss, dtype=np.float32)


def kernel(logits, targets):
    nc = _compiled()
    in_maps = make_in_maps(logits, targets)
    res = bass_utils.run_bass_kernel_spmd(nc, in_maps, core_ids=list(range(N_CORES)))
    return combine_outputs([r["out"] for r in res.results])
